# revision 2
# baseline (speedup 1.0000x reference)
"""Multi-head attention (B=4, S=1024, D=1024, H=16) on 8 Trainium2 NeuronCores.

Sharding (tensor-parallel over heads x data-parallel over batch):
core c handles batch b=c//2 and head-half hh=c%2 (8 of the 16 heads).
Each core projects Q/K/V for its 8 heads only (512 of the 1024 output
dims -- zero duplicated projection FLOPs), runs attention for its 8
heads over the full 1024x1024 score matrix, and computes the PARTIAL
output projection ctx_half @ Wo[:, half].T.  The host sums the two
partials per batch and adds bo during the gather -- no device
collectives.

All matmuls run as float32r (full-rate fp32 PE mode).  Layouts:
  qh^T, kh^T  [head_dim*2, S]   (transposed; contraction over head_dim)
  scores^T    [sk, sq]          (softmax over the partition dim sk)
  exp on ACT with per-partition mask bias; row-sums via a concurrent
  ones-column matmul (65-wide V stationary); normalization via
  gpsimd partition_broadcast of the reciprocal.
"""

import sys

for _p in ("/opt/trn_rl_repo", "/opt/pypackages"):
    if _p not in sys.path:
        sys.path.append(_p)

import numpy as np

B = 4
S = 1024
D = 1024
H = 16
HD = 64
HC = 8            # heads per core
DC = HC * HD      # 512 projection dims per core
KT = D // 128     # 8 contraction tiles (model dim)
CT = DC // 128    # 4 contraction tiles (core's ctx dims) = head pairs
SKT = S // 128    # 8 key tiles
NCORES = 8

_COMPILED = None


def _build():
    import concourse.bass as bass
    import concourse.mybir as mybir
    from concourse import bacc
    from concourse.bass import ts
    from concourse.tile import TileContext

    f32 = mybir.dt.float32
    f32r = mybir.dt.float32r
    i32 = mybir.dt.int32
    EXP = mybir.ActivationFunctionType.Exp

    nc = bacc.Bacc("TRN2", target_bir_lowering=False, debug=False,
                   num_devices=NCORES)

    xq_d = nc.dram_tensor("xq", [S, D], f32, kind="ExternalInput")
    xk_d = nc.dram_tensor("xk", [S, D], f32, kind="ExternalInput")
    xv_d = nc.dram_tensor("xv", [S, D], f32, kind="ExternalInput")
    mask_d = nc.dram_tensor("mask", [S], i32, kind="ExternalInput")
    wq_d = nc.dram_tensor("Wq", [DC, D], f32, kind="ExternalInput")
    wk_d = nc.dram_tensor("Wk", [DC, D], f32, kind="ExternalInput")
    wv_d = nc.dram_tensor("Wv", [DC, D], f32, kind="ExternalInput")
    wo_d = nc.dram_tensor("Wo", [D, DC], f32, kind="ExternalInput")
    bq_d = nc.dram_tensor("bq", [DC], f32, kind="ExternalInput")
    bk_d = nc.dram_tensor("bk", [DC], f32, kind="ExternalInput")
    bv_d = nc.dram_tensor("bv", [DC], f32, kind="ExternalInput")
    ones_d = nc.dram_tensor("ones", [128], f32, kind="ExternalInput")
    out_d = nc.dram_tensor("out", [S, D], f32, kind="ExternalOutput")

    with TileContext(nc) as tc:
        from contextlib import ExitStack
        with ExitStack() as stack:
            const = stack.enter_context(tc.tile_pool(name="const", bufs=1))
            vnat_p = stack.enter_context(tc.tile_pool(name="vnat", bufs=1))
            ctx_p = stack.enter_context(tc.tile_pool(name="ctxT", bufs=1))
            proj_ps = stack.enter_context(
                tc.tile_pool(name="proj_ps", bufs=2, space="PSUM"))
            scores_ps = stack.enter_context(
                tc.tile_pool(name="scores_ps", bufs=2, space="PSUM"))
            ctx_ps = stack.enter_context(
                tc.tile_pool(name="ctx_ps", bufs=1, space="PSUM"))

            # ---- constants -------------------------------------------------
            ones_sb = const.tile([128, 1], f32r, tag="ones")
            nc.sync.dma_start(ones_sb[:], ones_d[:].rearrange(
                "(a b) -> a b", b=1).bitcast(f32r))

            maskb = []
            for t in range(SKT):
                mi = const.tile([128, 1], i32, tag=f"mi{t}")
                nc.sync.dma_start(mi[:], mask_d[ts(t, 128)].rearrange(
                    "(a b) -> a b", b=1))
                mf = const.tile([128, 1], f32, tag=f"mf{t}")
                nc.vector.tensor_copy(mf[:], mi[:])
                mb = const.tile([128, 1], f32, tag=f"mb{t}")
                # (mask - 1) * 1e9  ->  0 for keep, -1e9 for masked
                nc.vector.tensor_scalar(mb[:], mf[:], 1e9, -1e9,
                                        mybir.AluOpType.mult,
                                        mybir.AluOpType.add)
                maskb.append(mb)

            def bias_col(d_handle, name, m):
                t = const.tile([128, 1], f32, tag=f"{name}{m}")
                nc.sync.dma_start(t[:], d_handle[ts(m, 128)].rearrange(
                    "(a b) -> a b", b=1))
                return t

            bq_t = [bias_col(bq_d, "bq", m) for m in range(CT)]
            bk_t = [bias_col(bk_d, "bk", m) for m in range(CT)]
            bv_bc = const.tile([128, DC], f32, tag="bvbc")
            nc.sync.dma_start(
                bv_bc[:],
                bass.AP(tensor=bv_d, offset=0, ap=[[0, 128], [1, DC]]))

            # ---- V projection (natural layout: [s, dout]) ------------------
            with tc.tile_pool(name="xv", bufs=1) as xv_p, \
                 tc.tile_pool(name="wv", bufs=1) as wv_p:
                xv_t = []
                for k in range(KT):
                    t = xv_p.tile([128, S], f32r, tag=f"xv{k}")
                    nc.sync.dma_start(t[:], xv_d[:, ts(k, 128)].rearrange(
                        "s d -> d s").bitcast(f32r))
                    xv_t.append(t)
                wv_t = []
                for k in range(KT):
                    t = wv_p.tile([128, DC], f32r, tag=f"wv{k}")
                    nc.sync.dma_start(t[:], wv_d[:, ts(k, 128)].rearrange(
                        "o d -> d o").bitcast(f32r))
                    wv_t.append(t)

                vnat = [vnat_p.tile([128, HC * 65], f32r, tag=f"v{m}",
                                    name=f"vnat{m}")
                        for m in range(SKT)]
                for m in range(SKT):
                    vv = vnat[m][:].rearrange("p (h x) -> p h x", x=65)
                    nc.sync.dma_start(
                        vv[:, :, 64:65],
                        bass.AP(tensor=ones_d, offset=0,
                                ap=[[1, 128], [0, HC], [0, 1]]).bitcast(f32r))
                    ps = proj_ps.tile([128, 512], f32, tag="pp")
                    for k in range(KT):
                        nc.tensor.matmul(
                            ps[:], xv_t[k][:, ts(m, 128)], wv_t[k][:],
                            start=(k == 0), stop=(k == KT - 1))
                    nc.vector.tensor_add(
                        vv[:, :, 0:64],
                        ps[:].rearrange("p (h x) -> p h x", x=64),
                        bv_bc[:].rearrange("p (h x) -> p h x", x=64))

            # ---- per head-pair: K/Q projection + attention -----------------
            ctxT = [ctx_p.tile([128, S], f32r, tag=f"c{k}", name=f"ctxT{k}")
                    for k in range(CT)]

            with tc.tile_pool(name="xq", bufs=1) as xq_p, \
                 tc.tile_pool(name="xk", bufs=1) as xk_p, \
                 tc.tile_pool(name="wqk", bufs=1) as wqk_p, \
                 tc.tile_pool(name="qkT", bufs=2) as qkT_p, \
                 tc.tile_pool(name="e", bufs=2) as e_p, \
                 tc.tile_pool(name="nrm", bufs=2) as nrm_p:

                xq_t = []
                for k in range(KT):
                    t = xq_p.tile([128, S], f32r, tag=f"xq{k}")
                    nc.sync.dma_start(t[:], xq_d[:, ts(k, 128)].rearrange(
                        "s d -> d s").bitcast(f32r))
                    xq_t.append(t)
                xk_t = []
                for k in range(KT):
                    t = xk_p.tile([128, S], f32r, tag=f"xk{k}")
                    nc.sync.dma_start(t[:], xk_d[:, ts(k, 128)].rearrange(
                        "s d -> d s").bitcast(f32r))
                    xk_t.append(t)

                wk_t = []
                for k in range(KT):
                    t = wqk_p.tile([128, DC], f32r, tag=f"wk{k}",
                                   name=f"wkt{k}")
                    nc.sync.dma_start(t[:], wk_d[:, ts(k, 128)].rearrange(
                        "o d -> d o").bitcast(f32r))
                    wk_t.append(t)
                wq_t = []
                for k in range(KT):
                    t = wqk_p.tile([128, DC], f32r, tag=f"wq{k}",
                                   name=f"wqt{k}")
                    nc.sync.dma_start(t[:], wq_d[:, ts(k, 128)].rearrange(
                        "o d -> d o").bitcast(f32r))
                    wq_t.append(t)

                for hp in range(CT):
                    khT = qkT_p.tile([128, S], f32r, tag="khT")
                    for si in range(2):
                        ps = proj_ps.tile([128, 512], f32, tag="pp")
                        for k in range(KT):
                            nc.tensor.matmul(
                                ps[:], wk_t[k][:, ts(hp, 128)],
                                xk_t[k][:, ts(si, 512)],
                                start=(k == 0), stop=(k == KT - 1))
                        nc.vector.tensor_scalar_add(
                            khT[:, ts(si, 512)], ps[:], bk_t[hp][:])
                    qhT = qkT_p.tile([128, S], f32r, tag="qhT")
                    for si in range(2):
                        ps = proj_ps.tile([128, 512], f32, tag="pp")
                        for k in range(KT):
                            nc.tensor.matmul(
                                ps[:], wq_t[k][:, ts(hp, 128)],
                                xq_t[k][:, ts(si, 512)],
                                start=(k == 0), stop=(k == KT - 1))
                        nc.vector.tensor_scalar_add(
                            qhT[:, ts(si, 512)], ps[:], bq_t[hp][:])

                    # attention for heads a=2*hp (partitions 0:64) and
                    # b=2*hp+1 (partitions 64:128), one 512-col q-half at
                    # a time
                    a, b = 2 * hp, 2 * hp + 1
                    for qh in range(2):
                        psCa = ctx_ps.tile([128, 512], f32, tag="ca")
                        psCb = ctx_ps.tile([128, 512], f32, tag="cb")
                        for t in range(SKT):
                            psS = scores_ps.tile([128, 1024], f32, tag="s")
                            nc.tensor.matmul(
                                psS[:, 0:512], khT[0:64, ts(t, 128)],
                                qhT[0:64, ts(qh, 512)], start=True, stop=True)
                            nc.tensor.matmul(
                                psS[:, 512:1024], khT[64:128, ts(t, 128)],
                                qhT[64:128, ts(qh, 512)], start=True,
                                stop=True, tile_position=(64, 0))
                            eT = e_p.tile([128, 1024], f32r, tag="e")
                            nc.scalar.activation(eT[:], psS[:], EXP,
                                                 bias=maskb[t][:],
                                                 scale=1.0 / np.sqrt(HD))
                            st, sp = (t == 0), (t == SKT - 1)
                            nc.tensor.matmul(
                                psCa[0:65, :], vnat[t][:, ts(a, 65)],
                                eT[:, 0:512], start=st, stop=sp)
                            nc.tensor.matmul(
                                psCb[0:65, :], vnat[t][:, ts(b, 65)],
                                eT[:, 512:1024], start=st, stop=sp)

                        for half, psC in ((0, psCa), (1, psCb)):
                            rec = nrm_p.tile([1, 512], f32, tag=f"r{half}")
                            nc.vector.reciprocal(rec[:], psC[64:65, :])
                            bc = nrm_p.tile([64, 512], f32, tag=f"b{half}")
                            nc.gpsimd.partition_broadcast(bc[:], rec[:])
                            nc.vector.tensor_mul(
                                ctxT[hp][64 * half:64 * half + 64,
                                         ts(qh, 512)],
                                psC[0:64, :], bc[:])

            # ---- output projection (partial: contraction over DC=512) ------
            with tc.tile_pool(name="wo", bufs=1) as wo_p, \
                 tc.tile_pool(name="outT", bufs=3) as out_p:
                wo_t = []
                for k in range(CT):
                    t = wo_p.tile([128, D], f32r, tag=f"wo{k}",
                                  name=f"wot{k}")
                    nc.sync.dma_start(t[:], wo_d[:, ts(k, 128)].rearrange(
                        "o d -> d o").bitcast(f32r))
                    wo_t.append(t)
                for m in range(KT):
                    for si in range(2):
                        ps = proj_ps.tile([128, 512], f32, tag="pp")
                        for k in range(CT):
                            nc.tensor.matmul(
                                ps[:], wo_t[k][:, ts(m, 128)],
                                ctxT[k][:, ts(si, 512)],
                                start=(k == 0), stop=(k == CT - 1))
                        ot = out_p.tile([128, 512], f32, tag="o")
                        nc.vector.tensor_copy(ot[:], ps[:])
                        nc.sync.dma_start(
                            out_d[ts(si, 512), ts(m, 128)].rearrange(
                                "s d -> d s"), ot[:])

    nc.compile()
    return nc


def _get_compiled():
    global _COMPILED
    if _COMPILED is None:
        _COMPILED = _build()
    return _COMPILED


def _in_maps(q, k, v, mask, Wq, bq, Wk, bk, Wv, bv, Wo, bo):
    q = np.ascontiguousarray(np.asarray(q, dtype=np.float32))
    k = np.ascontiguousarray(np.asarray(k, dtype=np.float32))
    v = np.ascontiguousarray(np.asarray(v, dtype=np.float32))
    mask = np.ascontiguousarray(np.asarray(mask, dtype=np.int32))
    Wq = np.asarray(Wq, np.float32)
    Wk = np.asarray(Wk, np.float32)
    Wv = np.asarray(Wv, np.float32)
    Wo = np.asarray(Wo, np.float32)
    bq = np.asarray(bq, np.float32)
    bk = np.asarray(bk, np.float32)
    bv = np.asarray(bv, np.float32)
    ones = np.ones((128,), np.float32)
    in_maps = []
    for c in range(NCORES):
        bidx, hh = c // 2, c % 2
        sl = slice(hh * DC, (hh + 1) * DC)
        in_maps.append({
            "xq": q[bidx],
            "xk": k[bidx],
            "xv": v[bidx],
            "mask": mask[bidx, 0],
            "Wq": np.ascontiguousarray(Wq[sl, :]),
            "Wk": np.ascontiguousarray(Wk[sl, :]),
            "Wv": np.ascontiguousarray(Wv[sl, :]),
            "Wo": np.ascontiguousarray(Wo[:, sl]),
            "bq": np.ascontiguousarray(bq[sl]),
            "bk": np.ascontiguousarray(bk[sl]),
            "bv": np.ascontiguousarray(bv[sl]),
            "ones": ones,
        })
    return in_maps


def _gather(results, bo):
    bo = np.asarray(bo, np.float32)
    out = np.empty((B, S, D), np.float32)
    for bidx in range(B):
        out[bidx] = results[2 * bidx]["out"]
        out[bidx] += results[2 * bidx + 1]["out"]
        out[bidx] += bo
    return out


def kernel(q, k, v, mask, Wq, bq, Wk, bk, Wv, bv, Wo, bo, **_ignored):
    from concourse.bass_utils import run_bass_kernel_spmd

    nc = _get_compiled()
    in_maps = _in_maps(q, k, v, mask, Wq, bq, Wk, bk, Wv, bv, Wo, bo)
    res = run_bass_kernel_spmd(nc, in_maps, core_ids=list(range(NCORES)))
    return _gather(res.results, bo)


# revision 9
# speedup vs baseline: 18.3454x; 18.3454x over previous
"""Multi-head attention (B=4, S=1024, D=1024, H=16) on 8 Trainium2 NeuronCores.

Sharding (tensor-parallel over heads x data-parallel over batch):
core c handles batch b=c//2 and head-half hh=c%2 (8 of the 16 heads).
Each core projects Q/K/V for its 8 heads only (512 of the 1024 output
dims -- zero duplicated projection FLOPs), runs attention for its 8
heads over the full 1024x1024 score matrix, and computes the PARTIAL
output projection ctx_half @ Wo[:, half].T.  The host sums the two
partials per batch and adds bo during the gather -- no device
collectives.

All device DMA is contiguous: activations and weights are pre-transposed
on the host ([model_dim, seq] / [d_in, d_out] layouts) and the output is
stored as out^T [d_out, seq]; the host transposes it back.  Strided
(4-byte-granular) DMA transposes cost ~5ms/core in the previous version.

All matmuls run as float32r (full-rate fp32 PE mode).  Layouts:
  qh^T, kh^T  [head_dim*2, S]   (contraction over head_dim)
  scores^T    [sk, sq]          (softmax over the partition dim sk)
  exp on ACT with per-partition mask bias; row-sums via a concurrent
  ones-column matmul (65-wide V stationary); normalization via
  gpsimd partition_broadcast of the reciprocal.
"""

import sys

for _p in ("/opt/trn_rl_repo", "/opt/pypackages"):
    if _p not in sys.path:
        sys.path.append(_p)

import numpy as np

B = 4
S = 1024
D = 1024
H = 16
HD = 64
HC = 8            # heads per core
DC = HC * HD      # 512 projection dims per core
KT = D // 128     # 8 contraction tiles (model dim)
CT = DC // 128    # 4 contraction tiles (core's ctx dims) = head pairs
SKT = S // 128    # 8 key tiles
NCORES = 8

_COMPILED = None


def _build():
    import concourse.bass as bass
    import concourse.mybir as mybir
    from concourse import bacc
    from concourse.bass import ts
    from concourse.tile import TileContext

    f32 = mybir.dt.float32
    f32r = mybir.dt.float32r
    i32 = mybir.dt.int32
    EXP = mybir.ActivationFunctionType.Exp

    nc = bacc.Bacc("TRN2", target_bir_lowering=False, debug=False,
                   num_devices=NCORES)

    # all activations/weights arrive pre-transposed, contiguous
    xq_d = nc.dram_tensor("xq", [D, S], f32, kind="ExternalInput")
    xk_d = nc.dram_tensor("xk", [D, S], f32, kind="ExternalInput")
    xv_d = nc.dram_tensor("xv", [D, S], f32, kind="ExternalInput")
    mask_d = nc.dram_tensor("mask", [S], i32, kind="ExternalInput")
    wq_d = nc.dram_tensor("Wq", [D, DC], f32, kind="ExternalInput")
    wk_d = nc.dram_tensor("Wk", [D, DC], f32, kind="ExternalInput")
    wv_d = nc.dram_tensor("Wv", [D, DC], f32, kind="ExternalInput")
    wo_d = nc.dram_tensor("Wo", [DC, D], f32, kind="ExternalInput")
    bq_d = nc.dram_tensor("bq", [DC], f32, kind="ExternalInput")
    bk_d = nc.dram_tensor("bk", [DC], f32, kind="ExternalInput")
    bv_d = nc.dram_tensor("bv", [DC], f32, kind="ExternalInput")
    ones_d = nc.dram_tensor("ones", [128], f32, kind="ExternalInput")
    out_d = nc.dram_tensor("out", [D, S], f32, kind="ExternalOutput")

    with TileContext(nc) as tc:
        from contextlib import ExitStack
        with ExitStack() as stack:
            const = stack.enter_context(tc.tile_pool(name="const", bufs=1))
            vnat_p = stack.enter_context(tc.tile_pool(name="vnat", bufs=1))
            ctx_p = stack.enter_context(tc.tile_pool(name="ctxT", bufs=1))
            proj_ps = stack.enter_context(
                tc.tile_pool(name="proj_ps", bufs=2, space="PSUM"))
            scores_ps = stack.enter_context(
                tc.tile_pool(name="scores_ps", bufs=2, space="PSUM"))
            ctx_ps = stack.enter_context(
                tc.tile_pool(name="ctx_ps", bufs=1, space="PSUM"))

            # ---- constants -------------------------------------------------
            maskb = []
            for t in range(SKT):
                mi = const.tile([128, 1], i32, tag=f"mi{t}")
                nc.sync.dma_start(mi[:], mask_d[ts(t, 128)].rearrange(
                    "(a b) -> a b", b=1))
                mf = const.tile([128, 1], f32, tag=f"mf{t}")
                nc.vector.tensor_copy(mf[:], mi[:])
                mb = const.tile([128, 1], f32, tag=f"mb{t}")
                # (mask - 1) * 1e9  ->  0 for keep, -1e9 for masked
                nc.vector.tensor_scalar(mb[:], mf[:], 1e9, -1e9,
                                        mybir.AluOpType.mult,
                                        mybir.AluOpType.add)
                maskb.append(mb)

            def bias_col(d_handle, name, m):
                t = const.tile([128, 1], f32, tag=f"{name}{m}")
                nc.sync.dma_start(t[:], d_handle[ts(m, 128)].rearrange(
                    "(a b) -> a b", b=1))
                return t

            bq_t = [bias_col(bq_d, "bq", m) for m in range(CT)]
            bk_t = [bias_col(bk_d, "bk", m) for m in range(CT)]
            bv_bc = const.tile([128, DC], f32, tag="bvbc")
            nc.sync.dma_start(
                bv_bc[:],
                bass.AP(tensor=bv_d, offset=0, ap=[[0, 128], [1, DC]]))

            # ---- V projection (natural layout: [s, dout]) ------------------
            with tc.tile_pool(name="xv", bufs=1) as xv_p, \
                 tc.tile_pool(name="wv", bufs=1) as wv_p:
                xv_t = []
                for k in range(KT):
                    t = xv_p.tile([128, S], f32r, tag=f"xv{k}")
                    nc.sync.dma_start(t[:], xv_d[ts(k, 128), :].bitcast(f32r))
                    xv_t.append(t)
                wv_t = []
                for k in range(KT):
                    t = wv_p.tile([128, DC], f32r, tag=f"wv{k}")
                    nc.sync.dma_start(t[:], wv_d[ts(k, 128), :].bitcast(f32r))
                    wv_t.append(t)

                vnat = [vnat_p.tile([128, HC * 65], f32r, tag=f"v{m}",
                                    name=f"vnat{m}")
                        for m in range(SKT)]
                for m in range(SKT):
                    vv = vnat[m][:].rearrange("p (h x) -> p h x", x=65)
                    nc.sync.dma_start(
                        vv[:, :, 64:65],
                        bass.AP(tensor=ones_d, offset=0,
                                ap=[[1, 128], [0, HC], [0, 1]]).bitcast(f32r))
                    ps = proj_ps.tile([128, 512], f32, tag="pp")
                    for k in range(KT):
                        nc.tensor.matmul(
                            ps[:], xv_t[k][:, ts(m, 128)], wv_t[k][:],
                            start=(k == 0), stop=(k == KT - 1))
                    nc.vector.tensor_add(
                        vv[:, :, 0:64],
                        ps[:].rearrange("p (h x) -> p h x", x=64),
                        bv_bc[:].rearrange("p (h x) -> p h x", x=64))

            # ---- per head-pair: K/Q projection + attention -----------------
            ctxT = [ctx_p.tile([128, S], f32r, tag=f"c{k}", name=f"ctxT{k}")
                    for k in range(CT)]

            with tc.tile_pool(name="xq", bufs=1) as xq_p, \
                 tc.tile_pool(name="xk", bufs=1) as xk_p, \
                 tc.tile_pool(name="wqk", bufs=1) as wqk_p, \
                 tc.tile_pool(name="qkT", bufs=2) as qkT_p, \
                 tc.tile_pool(name="e", bufs=2) as e_p, \
                 tc.tile_pool(name="nrm", bufs=2) as nrm_p:

                xq_t = []
                for k in range(KT):
                    t = xq_p.tile([128, S], f32r, tag=f"xq{k}")
                    nc.sync.dma_start(t[:], xq_d[ts(k, 128), :].bitcast(f32r))
                    xq_t.append(t)
                xk_t = []
                for k in range(KT):
                    t = xk_p.tile([128, S], f32r, tag=f"xk{k}")
                    nc.sync.dma_start(t[:], xk_d[ts(k, 128), :].bitcast(f32r))
                    xk_t.append(t)

                wk_t = []
                for k in range(KT):
                    t = wqk_p.tile([128, DC], f32r, tag=f"wk{k}",
                                   name=f"wkt{k}")
                    nc.sync.dma_start(t[:], wk_d[ts(k, 128), :].bitcast(f32r))
                    wk_t.append(t)
                wq_t = []
                for k in range(KT):
                    t = wqk_p.tile([128, DC], f32r, tag=f"wq{k}",
                                   name=f"wqt{k}")
                    nc.sync.dma_start(t[:], wq_d[ts(k, 128), :].bitcast(f32r))
                    wq_t.append(t)

                for hp in range(CT):
                    khT = qkT_p.tile([128, S], f32r, tag="khT")
                    for si in range(2):
                        ps = proj_ps.tile([128, 512], f32, tag="pp")
                        for k in range(KT):
                            nc.tensor.matmul(
                                ps[:], wk_t[k][:, ts(hp, 128)],
                                xk_t[k][:, ts(si, 512)],
                                start=(k == 0), stop=(k == KT - 1))
                        nc.vector.tensor_scalar_add(
                            khT[:, ts(si, 512)], ps[:], bk_t[hp][:])
                    qhT = qkT_p.tile([128, S], f32r, tag="qhT")
                    for si in range(2):
                        ps = proj_ps.tile([128, 512], f32, tag="pp")
                        for k in range(KT):
                            nc.tensor.matmul(
                                ps[:], wq_t[k][:, ts(hp, 128)],
                                xq_t[k][:, ts(si, 512)],
                                start=(k == 0), stop=(k == KT - 1))
                        nc.vector.tensor_scalar_add(
                            qhT[:, ts(si, 512)], ps[:], bq_t[hp][:])

                    # attention for heads a=2*hp (partitions 0:64) and
                    # b=2*hp+1 (partitions 64:128), one 512-col q-half at
                    # a time
                    a, b = 2 * hp, 2 * hp + 1
                    for qh in range(2):
                        psCa = ctx_ps.tile([128, 512], f32, tag="ca")
                        psCb = ctx_ps.tile([128, 512], f32, tag="cb")
                        for t in range(SKT):
                            psS = scores_ps.tile([128, 1024], f32, tag="s")
                            nc.tensor.matmul(
                                psS[:, 0:512], khT[0:64, ts(t, 128)],
                                qhT[0:64, ts(qh, 512)], start=True, stop=True)
                            nc.tensor.matmul(
                                psS[:, 512:1024], khT[64:128, ts(t, 128)],
                                qhT[64:128, ts(qh, 512)], start=True,
                                stop=True, tile_position=(64, 0))
                            eT = e_p.tile([128, 1024], f32r, tag="e")
                            nc.scalar.activation(eT[:], psS[:], EXP,
                                                 bias=maskb[t][:],
                                                 scale=1.0 / np.sqrt(HD))
                            st, sp = (t == 0), (t == SKT - 1)
                            nc.tensor.matmul(
                                psCa[0:65, :], vnat[t][:, ts(a, 65)],
                                eT[:, 0:512], start=st, stop=sp)
                            nc.tensor.matmul(
                                psCb[0:65, :], vnat[t][:, ts(b, 65)],
                                eT[:, 512:1024], start=st, stop=sp)

                        for half, psC in ((0, psCa), (1, psCb)):
                            rec = nrm_p.tile([1, 512], f32, tag=f"r{half}")
                            nc.vector.reciprocal(rec[:], psC[64:65, :])
                            bc = nrm_p.tile([64, 512], f32, tag=f"b{half}")
                            nc.gpsimd.partition_broadcast(bc[:], rec[:])
                            nc.vector.tensor_mul(
                                ctxT[hp][64 * half:64 * half + 64,
                                         ts(qh, 512)],
                                psC[0:64, :], bc[:])

            # ---- output projection (partial: contraction over DC=512) ------
            with tc.tile_pool(name="wo", bufs=1) as wo_p, \
                 tc.tile_pool(name="outT", bufs=3) as out_p:
                wo_t = []
                for k in range(CT):
                    t = wo_p.tile([128, D], f32r, tag=f"wo{k}",
                                  name=f"wot{k}")
                    nc.sync.dma_start(t[:], wo_d[ts(k, 128), :].bitcast(f32r))
                    wo_t.append(t)
                for m in range(KT):
                    for si in range(2):
                        ps = proj_ps.tile([128, 512], f32, tag="pp")
                        for k in range(CT):
                            nc.tensor.matmul(
                                ps[:], wo_t[k][:, ts(m, 128)],
                                ctxT[k][:, ts(si, 512)],
                                start=(k == 0), stop=(k == CT - 1))
                        ot = out_p.tile([128, 512], f32, tag="o")
                        nc.vector.tensor_copy(ot[:], ps[:])
                        nc.sync.dma_start(
                            out_d[ts(m, 128), ts(si, 512)], ot[:])

    nc.compile()
    return nc


def _get_compiled():
    global _COMPILED
    if _COMPILED is None:
        _COMPILED = _build()
    return _COMPILED


def _in_maps(q, k, v, mask, Wq, bq, Wk, bk, Wv, bv, Wo, bo):
    q = np.asarray(q, dtype=np.float32)
    k = np.asarray(k, dtype=np.float32)
    v = np.asarray(v, dtype=np.float32)
    mask = np.ascontiguousarray(np.asarray(mask, dtype=np.int32))
    Wq = np.asarray(Wq, np.float32)
    Wk = np.asarray(Wk, np.float32)
    Wv = np.asarray(Wv, np.float32)
    Wo = np.asarray(Wo, np.float32)
    bq = np.asarray(bq, np.float32)
    bk = np.asarray(bk, np.float32)
    bv = np.asarray(bv, np.float32)

    qT = [np.ascontiguousarray(q[bi].T) for bi in range(B)]
    kT = [np.ascontiguousarray(k[bi].T) for bi in range(B)]
    vT = [np.ascontiguousarray(v[bi].T) for bi in range(B)]
    whh = []
    for hh in range(2):
        sl = slice(hh * DC, (hh + 1) * DC)
        whh.append({
            "Wq": np.ascontiguousarray(Wq[sl, :].T),
            "Wk": np.ascontiguousarray(Wk[sl, :].T),
            "Wv": np.ascontiguousarray(Wv[sl, :].T),
            "Wo": np.ascontiguousarray(Wo[:, sl].T),
            "bq": np.ascontiguousarray(bq[sl]),
            "bk": np.ascontiguousarray(bk[sl]),
            "bv": np.ascontiguousarray(bv[sl]),
            "ones": np.ones((128,), np.float32),
        })
    in_maps = []
    for c in range(NCORES):
        bidx, hh = c // 2, c % 2
        in_maps.append({
            "xq": qT[bidx],
            "xk": kT[bidx],
            "xv": vT[bidx],
            "mask": mask[bidx, 0],
            **whh[hh],
        })
    return in_maps


def _gather(results, bo):
    bo = np.asarray(bo, np.float32)
    out = np.empty((B, S, D), np.float32)
    for bidx in range(B):
        acc = results[2 * bidx]["out"] + results[2 * bidx + 1]["out"]
        out[bidx] = acc.T
        out[bidx] += bo
    return out


def kernel(q, k, v, mask, Wq, bq, Wk, bk, Wv, bv, Wo, bo, **_ignored):
    from concourse.bass_utils import run_bass_kernel_spmd

    nc = _get_compiled()
    in_maps = _in_maps(q, k, v, mask, Wq, bq, Wk, bk, Wv, bv, Wo, bo)
    res = run_bass_kernel_spmd(nc, in_maps, core_ids=list(range(NCORES)))
    return _gather(res.results, bo)
